# revision 1
# baseline (speedup 1.0000x reference)
"""Bass/Trainium2 kernel for nn_DFTLayer: out[b,f,k] = DFT_1024(x[b,f,:]).

reference: real = einsum('bfs,ks->bfk', x, wcos); imag = ... wsin
           out  = complex(real, -imag),  x: [16, 1024, 1024] f32.

Strategy (8 NeuronCores, data-parallel over batch, 2 batches/core):
  - wcos/wsin are symmetric (w[k,s] == w[s,k]), so x @ w.T == x @ w.
  - Hermitian symmetry (x real): out[k] = conj(out[N-k]). The device only
    computes freq cols k = 1..512; col 0 is a host row-sum, cols 513..1023
    are a host conjugate mirror.
  - Cosine/sine parity over s (DCT/DST fold): with u[s] = x[s] + x[N-s],
    v[s] = x[s] - x[N-s] (s = 1..511), u[0] = v[0] = x[0]:
        real[k] = (U @ wcos[0:512, k]) + (-1)^k x[512]   (x[512] term on host)
        imag[k] =  V @ wsin[0:512, k]
    This halves both the matmul work and the DFT-kernel DMA.
  - U/V are built on the DVE (negative-stride reversed operand), transposed
    on the PE (128x128 blocks, 4 per PSUM bank), copied to SBUF as
    float32r, then contracted in 4 chunk-matmuls per output at N=512.
  - float32r (FP22 multiply, FP32 accumulate) runs at 1 PE cycle/row:
    4x faster than true fp32, rel err ~1.3e-4.
"""

import sys

for _p in ("/opt/trn_rl_repo", "/root/.axon_site/_ro/trn_rl_repo"):
    if _p not in sys.path:
        sys.path.append(_p)

import numpy as np
from contextlib import ExitStack

N_CORES = 8
B, F_FULL, S = 16, 1024, 1024          # x: [B, F_FULL, S]
F = (B // N_CORES) * F_FULL            # 2048 rows per core
KD = 512                               # device computes freq cols 1..512
SH = 512                               # folded contraction length (s = 0..511)
N_FT = F // 128                        # 16 row tiles per core
N_SC = SH // 128                       # 4 contraction chunks after the fold

_CACHE = {}

# feature flags (bisect/perf tuning)
DEVICE_C0 = True        # col-0 row-sum on device (else host numpy)
STT_RE = False          # re copy fused with alt*x512 (else host correction)
SPLIT_LAST = False      # split last f_tile's output stores
UVT_SPLIT = False       # uvt copies one-per-engine (ACT+DVE) vs both ACT
IM_ON_SYNC = False      # im out-DMA on HWDGE (sync) for tail queue overlap
PT_BUFS = 3             # transpose PSUM group double/triple buffering
XT_BUFS = 2             # uvt tile pipeline depth


def _build():
    """Build + compile the per-core Bass program (cached)."""
    if "nc" in _CACHE:
        return _CACHE["nc"]

    from concourse import bacc, tile, mybir

    f32 = mybir.dt.float32
    f32r = mybir.dt.float32r

    nc = bacc.Bacc("TRN2", target_bir_lowering=False, debug=False)

    x_d = nc.dram_tensor("x", [F, S], f32, kind="ExternalInput")
    wc_d = nc.dram_tensor("wc", [SH, KD], f32, kind="ExternalInput")
    ws_d = nc.dram_tensor("ws", [SH, KD], f32, kind="ExternalInput")
    re_d = nc.dram_tensor("re", [F, KD], f32, kind="ExternalOutput")
    im_d = nc.dram_tensor("im", [F, KD], f32, kind="ExternalOutput")
    # freq col 0 (real part = full row-sum), packed [partition, f_tile]
    c0_d = nc.dram_tensor("c0", [128, N_FT], f32, kind="ExternalOutput")

    ident_d = nc.inline_tensor(np.eye(128, dtype=np.float32), name="ident")
    # alt[j] = (-1)^(j+1) for device col j <-> freq k = j+1 (x[512] term)
    alt_np = np.tile(np.where(np.arange(1, KD + 1) % 2 == 0, 1.0, -1.0)
                     .astype(np.float32), (128, 1))
    alt_d = nc.inline_tensor(alt_np, name="alt")

    with tile.TileContext(nc) as tc, ExitStack() as ctx:
        wpool = ctx.enter_context(tc.tile_pool(name="w", bufs=1))
        xpool = ctx.enter_context(tc.tile_pool(name="x", bufs=3))
        uvpool = ctx.enter_context(tc.tile_pool(name="uv", bufs=2))
        xtpool = ctx.enter_context(tc.tile_pool(name="xt", bufs=XT_BUFS))
        opool = ctx.enter_context(tc.tile_pool(name="o", bufs=3))
        ptpool = ctx.enter_context(tc.tile_pool(name="pt", bufs=PT_BUFS, space="PSUM"))
        prpool = ctx.enter_context(tc.tile_pool(name="pr", bufs=2, space="PSUM"))
        pipool = ctx.enter_context(tc.tile_pool(name="pi", bufs=2, space="PSUM"))

        # x row-tile loads; first two issued before anything else so the
        # fold/transpose pipeline starts while the DFT kernels stream in.
        x_ts = [None] * N_FT

        def load_x(ft):
            x_t = xpool.tile([128, S], f32, tag="x_t")
            nc.sync.dma_start(x_t[:], x_d[ft * 128:(ft + 1) * 128, :])
            x_ts[ft] = x_t

        load_x(0)
        load_x(1)

        ident = wpool.tile([128, 128], f32r)
        nc.sync.dma_start(ident[:], ident_d[:].bitcast(f32r))
        c0_acc = wpool.tile([128, N_FT], f32)   # col-0 row-sums, one col/f_tile
        x5_acc = wpool.tile([128, N_FT], f32)   # x[:, 512] stash, one col/f_tile

        # Folded DFT kernels (rows s = 0..511), resident for the whole
        # run; one tile + DMA per 128-row chunk, in consumption order.
        wc_r = wc_d[:].rearrange("(c p) j -> p c j", p=128).bitcast(f32r)
        ws_r = ws_d[:].rearrange("(c p) j -> p c j", p=128).bitcast(f32r)
        wc_ts, ws_ts = [], []
        for c in range(N_SC):
            wc_t = wpool.tile([128, KD], f32r, tag=f"wc{c}")
            nc.sync.dma_start(wc_t[:], wc_r[:, c, :])
            wc_ts.append(wc_t)
            ws_t = wpool.tile([128, KD], f32r, tag=f"ws{c}")
            nc.sync.dma_start(ws_t[:], ws_r[:, c, :])
            ws_ts.append(ws_t)
        if STT_RE:
            alt_t = wpool.tile([128, KD], f32)
            nc.sync.dma_start(alt_t[:], alt_d[:])

        uvts = [None] * N_FT

        def fold_and_transpose(ft):
            x_t = x_ts[ft]
            # u = x[s] + x[1024-s], v = x[s] - x[1024-s]  (s = 1..511);
            # col 0 carries x[0] (cos row 0 == 1, sin row 0 == 0).
            # The U add also accumulates sum_{s=1..511} u[s] (accum_out),
            # from which freq col 0 = accum + x[0] + x[512].
            u_t = uvpool.tile([128, SH], f32r, tag="u")
            nc.vector.tensor_copy(u_t[:, 0:1], x_t[:, 0:1])
            nc.vector.tensor_add(u_t[:, 1:SH], x_t[:, 1:SH], x_t[:, S - 1:SH:-1])
            v_t = uvpool.tile([128, SH], f32r, tag="v")
            nc.vector.tensor_copy(v_t[:, 0:1], x_t[:, 0:1])
            nc.vector.tensor_sub(v_t[:, 1:SH], x_t[:, 1:SH], x_t[:, S - 1:SH:-1])
            # col-0 bookkeeping, off the PE-critical fold path:
            # c0 = sum_s u[s] + x[512] (u[0] already carries x[0]); stash
            # x[:, 512] for the fold edge term applied during the re copy.
            if DEVICE_C0:
                c0p = uvpool.tile([128, 1], f32, tag="c0p")
                nc.vector.reduce_sum(c0p[:], u_t[:].bitcast(f32),
                                     axis=mybir.AxisListType.X)
                nc.gpsimd.tensor_add(c0_acc[:, ft:ft + 1], c0p[:],
                                     x_t[:, 512:513])
            if STT_RE:
                nc.gpsimd.tensor_copy(x5_acc[:, ft:ft + 1], x_t[:, 512:513])
            # transpose U and V 128 cols at a time: uvt[:, c, :] holds
            # U chunks (c = 0..3) then V chunks (c = 4..7)
            uvt = xtpool.tile([128, 2 * N_SC, 128], f32r)
            for g, src in ((0, u_t), (1, v_t)):
                pt = ptpool.tile([128, N_SC, 128], f32r)
                for c in range(N_SC):
                    nc.tensor.matmul(
                        pt[:, c, :],
                        src[:, c * 128:(c + 1) * 128],
                        ident[:],
                        is_transpose=True,
                        start=(c == 0),
                        stop=(c == N_SC - 1),
                    )
                if g == 0:
                    nc.scalar.copy(uvt[:, 0:N_SC, :], pt[:])
                elif UVT_SPLIT:
                    nc.vector.tensor_copy(uvt[:, N_SC:2 * N_SC, :], pt[:])
                else:
                    nc.scalar.copy(uvt[:, N_SC:2 * N_SC, :], pt[:])
            uvts[ft] = uvt

        def matmul_and_store(ft):
            uvt = uvts[ft]
            ps_re = prpool.tile([128, KD], f32)
            for c in range(N_SC):
                nc.tensor.matmul(ps_re[:], uvt[:, c, :], wc_ts[c][:],
                                 start=(c == 0), stop=(c == N_SC - 1))
            ps_im = pipool.tile([128, KD], f32)
            for c in range(N_SC):
                nc.tensor.matmul(ps_im[:], uvt[:, N_SC + c, :], ws_ts[c][:],
                                 start=(c == 0), stop=(c == N_SC - 1))
            # real with the fold edge term: re = ps_re + alt * x[:, 512]
            nsplit = 2 if (SPLIT_LAST and ft == N_FT - 1) else 1
            w = KD // nsplit
            re_sb = opool.tile([128, KD], f32)
            im_sb = opool.tile([128, KD], f32)
            for h in range(nsplit):
                sl = slice(h * w, (h + 1) * w)
                if STT_RE:
                    nc.vector.scalar_tensor_tensor(
                        re_sb[:, sl], alt_t[:, sl], x5_acc[:, ft:ft + 1],
                        ps_re[:, sl],
                        op0=mybir.AluOpType.mult, op1=mybir.AluOpType.add,
                    )
                else:
                    nc.vector.tensor_copy(re_sb[:, sl], ps_re[:, sl])
                nc.gpsimd.dma_start(re_d[ft * 128:(ft + 1) * 128, sl], re_sb[:, sl])
                # negate imag on the way out: out.imag = -(v @ wsin)
                nc.scalar.mul(im_sb[:, sl], ps_im[:, sl], -1.0)
                im_eng = nc.sync if IM_ON_SYNC else nc.gpsimd
                im_eng.dma_start(im_d[ft * 128:(ft + 1) * 128, sl], im_sb[:, sl])

        # Software pipeline: fold+transposes of ft+1 hit the PE queue
        # before the matmuls of ft, so the PE never waits on the
        # DVE/ACT fold+copy chain.
        fold_and_transpose(0)
        for ft in range(1, N_FT):
            if ft + 1 < N_FT:
                load_x(ft + 1)
            fold_and_transpose(ft)
            matmul_and_store(ft - 1)
        matmul_and_store(N_FT - 1)
        if DEVICE_C0:
            nc.gpsimd.dma_start(c0_d[:], c0_acc[:])

    nc.compile()
    _CACHE["nc"] = nc
    return nc


def kernel(x, wsin, wcos):
    from concourse.bass_utils import run_bass_kernel_spmd

    x = np.asarray(x, dtype=np.float32)
    wsin = np.asarray(wsin, dtype=np.float32)
    wcos = np.asarray(wcos, dtype=np.float32)

    nc = _build()

    # By symmetry w[k, s] == w[s, k]: rows 0..511, freq cols 1..512.
    wc = np.ascontiguousarray(wcos[0:SH, 1:KD + 1])
    ws = np.ascontiguousarray(wsin[0:SH, 1:KD + 1])

    bpc = B // N_CORES
    in_maps = [
        {"x": np.ascontiguousarray(x[c * bpc:(c + 1) * bpc].reshape(F, S)),
         "wc": wc, "ws": ws}
        for c in range(N_CORES)
    ]

    res = run_bass_kernel_spmd(
        nc, in_maps, core_ids=list(range(N_CORES)), **_CACHE.get("run_kwargs", {})
    )
    kernel.last_results = res

    out = np.empty((B, F_FULL, S), dtype=np.complex64)
    fv = out.view(np.float32).reshape(B, F_FULL, 2 * S)
    for c in range(N_CORES):
        b0 = c * bpc
        re = res.results[c]["re"].reshape(bpc, F_FULL, KD)
        im = res.results[c]["im"].reshape(bpc, F_FULL, KD)  # already -imag
        blk = fv[b0:b0 + bpc]
        # col 0: real = row-sum of x (cos(0)=1), imag = 0 (sin(0)=0);
        # c0 is packed [partition, f_tile] -> row 128*ft + p
        if DEVICE_C0:
            blk[:, :, 0] = res.results[c]["c0"].T.reshape(bpc, F_FULL)
        else:
            blk[:, :, 0] = x[b0:b0 + bpc].sum(axis=-1, dtype=np.float32)
        blk[:, :, 1] = 0.0
        blk[:, :, 2:2 * KD + 2:2] = re          # real, k = 1..512
        blk[:, :, 3:2 * KD + 3:2] = im          # imag, k = 1..512
        # Hermitian mirror: out[k] = conj(out[1024-k]) for k = 513..1023
        blk[:, :, 2 * KD + 2::2] = re[:, :, KD - 2::-1]
        blk[:, :, 2 * KD + 3::2] = -im[:, :, KD - 2::-1]
    if not STT_RE:
        # the s = 512 fold edge term: real[k] += (-1)^k * x[:, :, 512]
        alt = np.where(np.arange(1, S) % 2 == 0, np.float32(1.0), np.float32(-1.0))
        fv[:, :, 2::2] += x[:, :, 512:513] * alt[None, None, :]
    return out



# revision 2
# speedup vs baseline: 1.5273x; 1.5273x over previous
"""Bass/Trainium2 kernel for nn_DFTLayer: out[b,f,k] = DFT_1024(x[b,f,:]).

reference: real = einsum('bfs,ks->bfk', x, wcos); imag = ... wsin
           out  = complex(real, -imag),  x: [16, 1024, 1024] f32.

Strategy (8 NeuronCores, data-parallel over batch, 2 batches/core):
  - Hermitian symmetry (x real): out[k] = conj(out[N-k]); device covers
    k = 0..255 directly and k = 257..512 via the radix-2 butterfly below;
    col 256 and the k = 513..1023 mirror are host-side.
  - Cosine/sine parity fold (host): u[s] = x[s] + x[N-s], v[s] = x[s] - x[N-s]
    over contraction slots s = 1..512 (u[512] = x[512], v[512] coeff is 0):
        real[k] = x[0] + sum_{s=1..512} u[s] cos(2*pi*k*s/N)
        imag[k] =        sum_{s=1..511} v[s] sin(2*pi*k*s/N)
  - Radix-2 split by parity of s (host): with ue[t] = u[2t+2], uo[t] = u[2t+1]
    (t = 0..255) and likewise ve/vo:
        E[k]  = ue @ wE[:,k],  O[k]  = uo @ wO[:,k]   (cos kernels)
        Es[k] = ve @ wEs[:,k], Os[k] = vo @ wOs[:,k]  (sin kernels)
        real[k]     = x[0] + E[k] + O[k]        k = 0..255
        real[512-k] = x[0] + E[k] - O[k]
        imag[k]     = Es[k] + Os[k],  imag[512-k] = -Es[k] + Os[k]
    This quarters the device matmul work vs the plain folded DFT.
  - Everything crossing HBM is bf16 (inputs pre-folded/transposed/cast on
    host, outputs cast bf16 on the way out): ~8.5 MB per core vs 18 MB for
    the f32 folded version; rel err ~4e-3, well under the 2e-2 gate.
  - Device program: w kernels stationary (16 [128,128] bf16 tiles), moving
    operand is the transposed fold data in 512-wide streams; 64 matmuls,
    32 PSUM->SBUF bf16 casts (split across ACT/DVE), 18 big DMAs.
    All butterflies/mirrors/corrections happen on the host.
"""

import sys

for _p in ("/opt/trn_rl_repo", "/root/.axon_site/_ro/trn_rl_repo"):
    if _p not in sys.path:
        sys.path.append(_p)

import numpy as np
import ml_dtypes
from contextlib import ExitStack

BF16 = np.dtype(ml_dtypes.bfloat16)

N_CORES = 8
B, F_FULL, S = 16, 1024, 1024          # x: [B, F_FULL, S]
F = (B // N_CORES) * F_FULL            # 2048 rows per core
M = 256                                # radix-2 contraction length
KD = 256                               # device freq cols per kernel (k = 0..255)
N_G = F // 512                         # 4 moving-operand groups of 512 rows

_CACHE = {}


def _build():
    """Build + compile the per-core Bass program (cached)."""
    if "nc" in _CACHE:
        return _CACHE["nc"]

    from concourse import bacc, tile, mybir

    f32 = mybir.dt.float32
    bf16 = mybir.dt.bfloat16

    nc = bacc.Bacc("TRN2", target_bir_lowering=False, debug=False)

    # uv rows: (inp*2 + tc)*128 + p  with t = tc*128 + p, inp in (ue,uo,ve,vo)
    uv_d = nc.dram_tensor("uv", [8 * 128, F], bf16, kind="ExternalInput")
    # w rows: t (0..255); cols: (kern*2 + kc)*128 + q with k = kc*128 + q
    w_d = nc.dram_tensor("w", [M, 4 * KD], bf16, kind="ExternalInput")
    # eo rows: (kern*2 + kc)*128 + q  (freq k = kc*128 + q), cols: core rows
    eo_d = nc.dram_tensor("eo", [8 * 128, F], bf16, kind="ExternalOutput")

    with tile.TileContext(nc) as tc, ExitStack() as ctx:
        wpool = ctx.enter_context(tc.tile_pool(name="w", bufs=1))
        opool = ctx.enter_context(tc.tile_pool(name="o", bufs=3))
        ppool = ctx.enter_context(tc.tile_pool(name="p", bufs=2, space="PSUM"))

        # stationary DFT kernels: one tile per tc chunk, resident all run
        w_ts = []
        for t in range(2):
            w_t = wpool.tile([128, 4 * KD], bf16, tag=f"w{t}")
            nc.sync.dma_start(w_t[:], w_d[t * 128:(t + 1) * 128, :])
            w_ts.append(w_t)

        # fold data, transposed: 8 tiles [128, 2048], loaded in phase order
        uv_ts = []
        for i in range(8):
            uv_t = wpool.tile([128, F], bf16, tag=f"uv{i}")
            nc.sync.dma_start(uv_t[:], uv_d[i * 128:(i + 1) * 128, :])
            uv_ts.append(uv_t)

        for kern in range(4):
            for kc in range(2):
                ps = ppool.tile([128, N_G, 512], f32)
                for t in range(2):
                    lhsT = w_ts[t][:, (kern * 2 + kc) * 128:(kern * 2 + kc + 1) * 128]
                    for g in range(N_G):
                        nc.tensor.matmul(
                            ps[:, g, :],
                            lhsT,
                            uv_ts[kern * 2 + t][:, g * 512:(g + 1) * 512],
                            start=(t == 0),
                            stop=(t == 1),
                        )
                out_t = opool.tile([128, F], bf16)
                for g in range(N_G):
                    eng = nc.scalar if g % 2 == 0 else nc.vector
                    if g % 2 == 0:
                        nc.scalar.copy(out_t[:, g * 512:(g + 1) * 512], ps[:, g, :])
                    else:
                        nc.vector.tensor_copy(out_t[:, g * 512:(g + 1) * 512], ps[:, g, :])
                r0 = (kern * 2 + kc) * 128
                nc.gpsimd.dma_start(eo_d[r0:r0 + 128, :], out_t[:])

    nc.compile()
    _CACHE["nc"] = nc
    return nc


def kernel(x, wsin, wcos):
    from concourse.bass_utils import run_bass_kernel_spmd

    x = np.asarray(x, dtype=np.float32)
    wsin = np.asarray(wsin, dtype=np.float32)
    wcos = np.asarray(wcos, dtype=np.float32)

    nc = _build()

    # radix-2 DFT kernels, sliced from the provided (symmetric) matrices:
    #   wE[t,k] = cos(2*pi*k*(2t+2)/N), wO[t,k] = cos(2*pi*k*(2t+1)/N)
    wE = wcos[2:513:2, 0:KD]
    wO = wcos[1:512:2, 0:KD]
    wEs = wsin[2:513:2, 0:KD]
    wOs = wsin[1:512:2, 0:KD]
    w_np = np.ascontiguousarray(
        np.concatenate([wE, wO, wEs, wOs], axis=1)).astype(BF16)

    # host fold + parity split (f32), then bf16
    xa = x[:, :, 1:512]
    xb = x[:, :, 1023:512:-1]
    u = xa + xb                         # u[s], s = 1..511
    v = xa - xb
    uvp = np.empty((B, F_FULL, 4, M), dtype=np.float32)
    uvp[:, :, 0, :255] = u[:, :, 1::2]  # ue: s = 2,4,..,510
    uvp[:, :, 0, 255] = x[:, :, 512]    # ue[255] <- u[512] = x[512]
    uvp[:, :, 1, :] = u[:, :, 0::2]     # uo: s = 1,3,..,511
    uvp[:, :, 2, :255] = v[:, :, 1::2]  # ve
    uvp[:, :, 2, 255] = 0.0
    uvp[:, :, 3, :] = v[:, :, 0::2]     # vo
    uvp_bf = uvp.astype(BF16)

    bpc = B // N_CORES
    in_maps = []
    for c in range(N_CORES):
        blk = uvp_bf[c * bpc:(c + 1) * bpc].reshape(F, 4, M)
        uv_c = np.ascontiguousarray(blk.transpose(1, 2, 0)).reshape(8 * 128, F)
        in_maps.append({"uv": uv_c, "w": w_np})

    res = run_bass_kernel_spmd(
        nc, in_maps, core_ids=list(range(N_CORES)), **_CACHE.get("run_kwargs", {})
    )
    kernel.last_results = res

    # host assembly: butterflies, x[0] correction, col 256, Hermitian mirror
    alt = np.where(np.arange(M) % 2 == 0, np.float32(1.0), np.float32(-1.0))
    out = np.empty((B, F_FULL, S), dtype=np.complex64)
    fv = out.view(np.float32).reshape(B, F_FULL, 2 * S)
    for c in range(N_CORES):
        b0 = c * bpc
        eo = np.asarray(res.results[c]["eo"]).reshape(4, KD, F)
        E = eo[0].T.astype(np.float32)      # [F, KD]
        O = eo[1].T.astype(np.float32)
        Es = eo[2].T.astype(np.float32)
        Os = eo[3].T.astype(np.float32)
        x0 = x[b0:b0 + bpc, :, 0].reshape(F, 1)
        reA = E + O
        reA += x0
        reB = E - O
        reB += x0
        imA = Es + Os
        np.negative(imA, out=imA)           # out.imag = -imag_raw
        imB = Es - Os
        fvb = fv[b0:b0 + bpc].reshape(F, 2 * S)
        fvb[:, 0:2 * KD:2] = reA            # real, k = 0..255
        fvb[:, 1:2 * KD:2] = imA
        fvb[:, 514:1026:2] = reB[:, ::-1]   # real, k = 257..512
        fvb[:, 515:1027:2] = imB[:, ::-1]
        # col 256: even-s cos run is (-1)^(t+1), odd-s sin run is (-1)^t
        ue32 = uvp[b0:b0 + bpc, :, 0, :].reshape(F, M)
        vo32 = uvp[b0:b0 + bpc, :, 3, :].reshape(F, M)
        fvb[:, 512] = x0[:, 0] - ue32 @ alt
        fvb[:, 513] = -(vo32 @ alt)
        # Hermitian mirror: out[k] = conj(out[1024-k]) for k = 513..1023
        fvb[:, 1026::2] = fvb[:, 1022:0:-2]
        fvb[:, 1027::2] = -fvb[:, 1023:1:-2]
    return out
